# revision 1
# baseline (speedup 1.0000x reference)
"""Trainium2 Bass kernel for nn_LBLHighwayBiLm.

Model (hardcoded): L=2 layers x {fw,bw} directions. Per layer+direction:
  5-tap windowed sum along seq (with learned pad vectors), then a 2-deep
  AllenNLP Highway stack (H=1024 -> 2H proj, sigmoid gate + relu), residual
  from layer input for l>0. Output: [L, B, S, 2H] = concat(f, bw).

Strategy: data-parallel over batch across 8 NeuronCores (B=32 -> 4/core).
On-chip layout is feature-major [H(partitions), tokens(free)]:
  - window sums run on DVE as 5 scalar_tensor_tensor taps over a padded
    staging tile, accumulating straight into an fp32r tile,
  - highway projections run on PE as [128,128]x[128,512] fp32r matmuls
    accumulating over H in PSUM (weights are host-pre-rounded to fp32r,
    which is fp32 RNE-rounded to 11 mantissa bits, so they DMA directly
    into fp32r tiles),
  - bias+sigmoid / bias+relu fuse into single ACT ops reading PSUM,
  - the gate combine g*x + (1-g)*relu runs on DVE (3 ops).
Layer-0 outputs round-trip through a DRAM scratch pool for layer 1's
window sum + residual. Host pre-transposes inputs to feature-major and
re-assembles the [L, B, S, 2H] output from per-core [L, 2, H, B/8, S].
"""

import sys

for _p in ("/opt/trn_rl_repo", "/opt/pypackages"):
    if _p not in sys.path:
        sys.path.insert(0, _p)

import numpy as np

import concourse.bass as bass
import concourse.tile as tile
from concourse import mybir, bacc
from concourse import bass_utils

# Problem constants (hardcoded per contract)
L = 2
NH = 2
WIDTH = 4
H = 1024
B, S = 32, 512
CORES = 8
BL = B // CORES          # batch per core
KT = H // 128            # 8 contraction tiles
NT = 2 * H // 128        # 16 output feature tiles
PSEQ = S + 2 * WIDTH     # padded seq length 520

FP32 = mybir.dt.float32
FP32R = mybir.dt.float32r
AF = mybir.ActivationFunctionType
OP = mybir.AluOpType


def rne_round_fp32(x: np.ndarray, mbits: int = 11) -> np.ndarray:
    """Round fp32 to `mbits` explicit mantissa bits (RNE) — the fp32r format."""
    u = np.ascontiguousarray(x, dtype=np.float32).view(np.uint32).astype(np.uint64)
    shift = 23 - mbits
    bias = ((u >> shift) & 1) + ((1 << (shift - 1)) - 1)
    u = (u + bias) & ~np.uint64((1 << shift) - 1)
    return (u & 0xFFFFFFFF).astype(np.uint32).view(np.float32).reshape(x.shape)


def build_nc(loop_n: int = 1):
    """Build the per-core SPMD Bass program. Every core runs the same NEFF
    on its own batch shard (inputs differ, weights replicated).
    loop_n > 1 wraps the body in a hardware loop (timing amplification only)."""
    nc = bacc.Bacc("TRN2", target_bir_lowering=False, debug=False)

    x_t = nc.dram_tensor("x_t", [H, BL, S], FP32, kind="ExternalInput").ap()
    fwpad = nc.dram_tensor("fwpad", [L, H, WIDTH], FP32, kind="ExternalInput").ap()
    bwpad = nc.dram_tensor("bwpad", [L, H, WIDTH], FP32, kind="ExternalInput").ap()
    fw_w = nc.dram_tensor("fw_w", [L, WIDTH + 1], FP32, kind="ExternalInput").ap()
    bw_w = nc.dram_tensor("bw_w", [L, WIDTH + 1], FP32, kind="ExternalInput").ap()
    fw_W = nc.dram_tensor("fw_W", [L, NH, H, 2 * H], FP32R, kind="ExternalInput").ap()
    bw_W = nc.dram_tensor("bw_W", [L, NH, H, 2 * H], FP32R, kind="ExternalInput").ap()
    fw_b = nc.dram_tensor("fw_b", [L, NH, 2 * H], FP32, kind="ExternalInput").ap()
    bw_b = nc.dram_tensor("bw_b", [L, NH, 2 * H], FP32, kind="ExternalInput").ap()
    out = nc.dram_tensor("out", [L, 2, H, BL, S], FP32, kind="ExternalOutput").ap()

    with tile.TileContext(nc) as tc:
        if loop_n == 1:
            _emit(tc, nc, x_t, fwpad, bwpad, fw_w, bw_w, fw_W, bw_W, fw_b, bw_b, out)
        else:
            with tc.For_i(0, loop_n, 1):
                _emit(tc, nc, x_t, fwpad, bwpad, fw_w, bw_w, fw_W, bw_W,
                      fw_b, bw_b, out)
    nc.compile()
    return nc


def _emit(tc, nc, x_t, fwpad, bwpad, fw_w, bw_w, fw_W, bw_W, fw_b, bw_b, out):
    from contextlib import ExitStack
    ctx = ExitStack()
    ws_pool = ctx.enter_context(tc.tile_pool(name="ws", bufs=1))
    x0_pool = ctx.enter_context(tc.tile_pool(name="x0", bufs=1))
    stage_pool = ctx.enter_context(tc.tile_pool(name="stage", bufs=2))
    w_pool = ctx.enter_context(tc.tile_pool(name="wts", bufs=6))
    psum = ctx.enter_context(tc.tile_pool(name="psum", bufs=8, space="PSUM"))
    ract = ctx.enter_context(tc.tile_pool(name="ract", bufs=2))
    sact = ctx.enter_context(tc.tile_pool(name="sact", bufs=2))
    dtmp = ctx.enter_context(tc.tile_pool(name="dtmp", bufs=2))
    ostage = ctx.enter_context(tc.tile_pool(name="ostage", bufs=4))
    resp = ctx.enter_context(tc.tile_pool(name="resp", bufs=4))
    consts = ctx.enter_context(tc.tile_pool(name="consts", bufs=2))
    dram = ctx.enter_context(tc.tile_pool(name="dram", bufs=1, space="DRAM"))

    # layer-0 outputs (pre-concat) kept in DRAM scratch for layer-1 reads
    scr = [dram.tile([H, BL, S], FP32, tag=f"scr{d}", name=f"scr{d}")
           for d in range(2)]

    W_by_dir = (fw_W, bw_W)
    b_by_dir = (fw_b, bw_b)
    w_by_dir = (fw_w, bw_w)

    for l in range(L):
        for d in range(2):  # 0=fw, 1=bw
            # --- window-sum scalars, broadcast to [128, 5]
            wsrc = w_by_dir[d]
            wt = consts.tile([128, WIDTH + 1], FP32, tag="wt")
            nc.sync.dma_start(
                out=wt,
                in_=bass.AP(tensor=wsrc.tensor, offset=l * (WIDTH + 1),
                            ap=[[0, 128], [1, WIDTH + 1]]))

            # --- staging + windowed sum into fp32r ws
            ws = ws_pool.tile([128, KT, BL, S], FP32R, tag="ws")
            offs = 0 if d == 0 else WIDTH
            for kt in range(KT):
                stg = stage_pool.tile([128, BL, PSEQ], FP32, tag="stage")
                if l == 0:
                    body_src = x_t[bass.ts(kt, 128), :, :]
                else:
                    body_src = scr[d][bass.ts(kt, 128), :, :]
                nc.sync.dma_start(out=stg[:, :, WIDTH:WIDTH + S], in_=body_src)
                # NOTE: reference prepends fw_pad and appends bw_pad for BOTH
                # directions' padded sequences.
                fp_ap = fwpad[l, bass.ts(kt, 128), :]
                bp_ap = bwpad[l, bass.ts(kt, 128), :]
                nc.sync.dma_start(
                    out=stg[:, :, 0:WIDTH],
                    in_=bass.AP(tensor=fp_ap.tensor, offset=fp_ap.offset,
                                ap=[fp_ap.ap[0], [0, BL], fp_ap.ap[1]]))
                nc.sync.dma_start(
                    out=stg[:, :, WIDTH + S:PSEQ],
                    in_=bass.AP(tensor=bp_ap.tensor, offset=bp_ap.offset,
                                ap=[bp_ap.ap[0], [0, BL], bp_ap.ap[1]]))
                nc.vector.tensor_scalar(
                    ws[:, kt], stg[:, :, offs:offs + S], wt[:, 0:1], None,
                    op0=OP.mult)
                for k in range(1, WIDTH + 1):
                    nc.vector.scalar_tensor_tensor(
                        ws[:, kt], stg[:, :, offs + k:offs + k + S], wt[:, k:k + 1],
                        ws[:, kt], op0=OP.mult, op1=OP.add)

            # --- highway sublayer 0: ws -> x0 (fp32r)
            x0 = x0_pool.tile([128, KT, BL, S], FP32R, tag="x0")
            _highway(tc, nc, w_pool, psum, ract, sact, dtmp, consts,
                     W_by_dir[d], b_by_dir[d], l, 0, ws, x0=x0)

            # --- highway sublayer 1: x0 -> out (+residual for l>0)
            _highway(tc, nc, w_pool, psum, ract, sact, dtmp, consts,
                     W_by_dir[d], b_by_dir[d], l, 1, x0,
                     nc_out=out, ostage=ostage, resp=resp,
                     scr=scr[d] if l == 0 else None,
                     res_src=scr[d] if l > 0 else None, l_idx=l, d_idx=d)

    ctx.close()


def _highway(tc, nc, w_pool, psum, ract, sact, dtmp, consts,
             W_src, b_src, l, i, x_in, x0=None, nc_out=None, ostage=None,
             resp=None, scr=None, res_src=None, l_idx=None, d_idx=None):
    # bias [2H] -> [128, 16]; column n is features n*128..(n+1)*128
    bt = consts.tile([128, NT], FP32, tag="bt")
    b_ap = b_src[l, i, :]
    nc.sync.dma_start(out=bt, in_=b_ap.rearrange("(n p) -> p n", p=128))

    # weight source view [H, 2H] -> [p, kt, col]
    Wv = W_src[l, i].rearrange("(kt p) c -> p kt c", p=128)

    for jj in range(KT):  # paired feature tiles: nl=jj, gate=jj+8
        Wnl = w_pool.tile([128, KT, 128], FP32R, tag="W")
        nc.sync.dma_start(out=Wnl, in_=Wv[:, :, bass.ts(jj, 128)])
        Wsg = w_pool.tile([128, KT, 128], FP32R, tag="W")
        nc.sync.dma_start(out=Wsg, in_=Wv[:, :, bass.ts(jj + KT, 128)])
        for t in range(BL):
            ps_nl = psum.tile([128, S], FP32, tag="ps")
            ps_sg = psum.tile([128, S], FP32, tag="ps")
            for k in range(KT):
                nc.tensor.matmul(ps_nl, Wnl[:, k], x_in[:, k, t],
                                 start=(k == 0), stop=(k == KT - 1))
            for k in range(KT):
                nc.tensor.matmul(ps_sg, Wsg[:, k], x_in[:, k, t],
                                 start=(k == 0), stop=(k == KT - 1))
            r = ract.tile([128, S], FP32, tag="r")
            nc.scalar.activation(r, ps_nl, AF.Relu, bias=bt[:, jj:jj + 1])
            sg = sact.tile([128, S], FP32, tag="sg")
            nc.scalar.activation(sg, ps_sg, AF.Sigmoid, bias=bt[:, jj + KT:jj + KT + 1])
            dd = dtmp.tile([128, S], FP32, tag="dd")
            nc.vector.tensor_sub(dd, x_in[:, jj, t], r)   # x - relu
            nc.vector.tensor_mul(dd, sg, dd)              # g*(x - relu)
            if x0 is not None:
                # out = g*x + (1-g)*relu = g*(x-relu) + relu
                nc.vector.tensor_add(x0[:, jj, t], dd, r)
            else:
                ot = ostage.tile([128, S], FP32, tag="ot")
                if res_src is None:
                    nc.vector.tensor_add(ot, dd, r)
                else:
                    rs = resp.tile([128, S], FP32, tag="rs")
                    nc.sync.dma_start(out=rs, in_=res_src[bass.ts(jj, 128), t, :])
                    nc.vector.tensor_add(dd, dd, r)
                    nc.vector.tensor_add(ot, dd, rs)      # + layer input (residual)
                nc.sync.dma_start(out=nc_out[l_idx, d_idx, bass.ts(jj, 128), t, :],
                                  in_=ot)
                if scr is not None:
                    nc.sync.dma_start(out=scr[bass.ts(jj, 128), t, :], in_=ot)


def prepare_in_maps(inputs, fw_pad, bw_pad, fw_w, bw_w,
                    fw_hw_W, fw_hw_b, bw_hw_W, bw_hw_b):
    """Shard + lay out host-side. Returns list of 8 per-core input dicts."""
    fw_W_r = rne_round_fp32(fw_hw_W)
    bw_W_r = rne_round_fp32(bw_hw_W)
    fwpad_t = np.ascontiguousarray(np.transpose(
        np.asarray(fw_pad, dtype=np.float32), (0, 2, 1)))
    bwpad_t = np.ascontiguousarray(np.transpose(
        np.asarray(bw_pad, dtype=np.float32), (0, 2, 1)))
    common = {
        "fwpad": fwpad_t, "bwpad": bwpad_t,
        "fw_w": np.ascontiguousarray(fw_w, dtype=np.float32),
        "bw_w": np.ascontiguousarray(bw_w, dtype=np.float32),
        "fw_W": fw_W_r, "bw_W": bw_W_r,
        "fw_b": np.ascontiguousarray(fw_hw_b, dtype=np.float32),
        "bw_b": np.ascontiguousarray(bw_hw_b, dtype=np.float32),
    }
    in_maps = []
    for c in range(CORES):
        shard = np.asarray(inputs[c * BL:(c + 1) * BL], dtype=np.float32)
        x_feat = np.ascontiguousarray(np.transpose(shard, (2, 0, 1)))  # [H, BL, S]
        in_maps.append({"x_t": x_feat, **common})
    return in_maps


def assemble_output(results):
    """Per-core out [L, 2, H, BL, S] -> full [L, B, S, 2H]."""
    full = np.empty((L, B, S, 2 * H), dtype=np.float32)
    for c, r in enumerate(results):
        o = r["out"]  # [L, 2, H, BL, S]
        # [L, 2, H, BL, S] -> [L, BL, S, 2, H] -> [L, BL, S, 2H]
        full[:, c * BL:(c + 1) * BL] = np.transpose(
            o, (0, 3, 4, 1, 2)).reshape(L, BL, S, 2 * H)
    return full


_NC_CACHE = None


def kernel(inputs, masks, fw_pad, bw_pad, fw_w, bw_w,
           fw_hw_W, fw_hw_b, bw_hw_W, bw_hw_b):
    del masks  # all-ones; unused by the reference computation
    global _NC_CACHE
    if _NC_CACHE is None:
        _NC_CACHE = build_nc()
    in_maps = prepare_in_maps(inputs, fw_pad, bw_pad, fw_w, bw_w,
                              fw_hw_W, fw_hw_b, bw_hw_W, bw_hw_b)
    res = bass_utils.run_bass_kernel_spmd(_NC_CACHE, in_maps,
                                          core_ids=list(range(CORES)))
    return assemble_output(res.results)


if __name__ == "__main__":
    nc = build_nc()
    print("built ok")



# revision 4
# speedup vs baseline: 1.5111x; 1.5111x over previous
"""Trainium2 Bass kernel for nn_LBLHighwayBiLm — v4 (fp16 + ldweights).

Model (hardcoded): L=2 layers x {fw,bw} directions. Per layer+direction:
  5-tap windowed sum along seq (learned pad vectors), then a 2-deep AllenNLP
  Highway stack (H=1024 -> 2H proj, sigmoid gate + relu), residual from the
  layer input for l>0. Output: [L, B, S, 2H] = concat(f, bw).

Strategy: data-parallel over batch across 8 cores (B=32 -> 4/core),
feature-major on-chip layout [H(part), tokens(free)], fp16 pipeline
(measured end-to-end rel err ~2.6e-3; PE fp16 streams at the full
2.8 GHz col/cycle when weights are loaded via explicit ldweights).

Performance structure (HW-measured numbers):
  - Matmuls run k-outer / t-inner with an explicit ldweights per k-tile,
    so each stationary load is amortized over 4 token blocks: 183 ns per
    [128,128]x[128,512] matmul (vs 276 self-loading) -> 750 us PE floor.
  - Window sums are a tensor_scalar (4x packed mode) + tensor_add (2x)
    tree on DVE; scalar_tensor_tensor only runs 1x so it is avoided.
  - The layer-0 fw window sum gates the whole pipeline, so ~60% of it is
    computed on the otherwise-idle PE as 5 accumulating diagonal matmuls
    (diag(w_k) stationary, built host-side), ACT copying PSUM -> fp16.
  - Every combine's final add runs on Pool (tensor_tensor is legal there),
    keeping DVE inside the per-phase window (ws tree + sub/mul).
  - Layer-1 staging lives in SBUF: highway-1 combines write straight into
    the padded staging tile; the l=1 residual reads the same body. No DRAM
    scratch round-trip.
  - Sublayer order per layer is h0_f, h0_b, h1_f, h1_b so each PE phase's
    inputs are ready >=1 phase ahead; ws tiles and x0 tiles share one
    3-buffer pool (lifetimes interleave exactly).
"""

import sys

for _p in ("/opt/trn_rl_repo", "/opt/pypackages"):
    if _p not in sys.path:
        sys.path.insert(0, _p)

import numpy as np

import concourse.bass as bass
import concourse.tile as tile
from concourse import mybir, bacc
from concourse import bass_utils

# Problem constants (hardcoded per contract)
L = 2
NH = 2
WIDTH = 4
H = 1024
B, S = 32, 512
CORES = 8
BL = B // CORES          # batch per core
KT = H // 128            # 8 contraction tiles
NT = 2 * H // 128        # 16 output feature tiles
PSEQ = S + 2 * WIDTH     # padded seq length 520
NSCAL = 4 * (WIDTH + 1) + 8 * NT   # packed scalars: 4 tap rows + 8 bias rows
PE_WS_KT = 5             # prologue ws_f0: kt 0..PE_WS_KT-1 via PE diag matmuls

FP32 = mybir.dt.float32
FP16 = mybir.dt.float16
AF = mybir.ActivationFunctionType
OP = mybir.AluOpType


def build_nc(loop_n: int = 1):
    """Per-core SPMD Bass program; every core runs the same NEFF on its own
    batch shard (weights replicated). loop_n > 1 wraps the body in a hardware
    loop (timing amplification only)."""
    nc = bacc.Bacc("TRN2", target_bir_lowering=False, debug=False)

    # [H, BL, PSEQ] layer-0 padded staging, pads baked in host-side
    xstage = nc.dram_tensor("xstage", [H, BL, PSEQ], FP16, kind="ExternalInput").ap()
    # weights pre-tiled host-side: [dir, L, NH, NT, 128, KT, 128]
    wts = nc.dram_tensor("wts", [2, L, NH, NT, 128, KT, 128], FP16,
                         kind="ExternalInput").ap()
    # layer-1 pad vectors, feature-major [H, WIDTH]
    fwpad1 = nc.dram_tensor("fwpad1", [H, WIDTH], FP16, kind="ExternalInput").ap()
    bwpad1 = nc.dram_tensor("bwpad1", [H, WIDTH], FP16, kind="ExternalInput").ap()
    # diag(w_k) for the layer-0 fw window taps (PE-side window sum)
    diagw = nc.dram_tensor("diagw", [128, WIDTH + 1, 128], FP16,
                           kind="ExternalInput").ap()
    # all per-(l,d[,i]) scalars packed into one [128, NSCAL] fp32 blob
    scal = nc.dram_tensor("scal", [128, NSCAL], FP32, kind="ExternalInput").ap()
    out = nc.dram_tensor("out", [L, 2, H, BL, S], FP16, kind="ExternalOutput").ap()

    with tile.TileContext(nc) as tc:
        if loop_n == 1:
            _emit(tc, nc, xstage, wts, fwpad1, bwpad1, diagw, scal, out)
        else:
            with tc.For_i(0, loop_n, 1):
                _emit(tc, nc, xstage, wts, fwpad1, bwpad1, diagw, scal, out)
    nc.compile()
    return nc


def _emit(tc, nc, xstage, wts, fwpad1, bwpad1, diagw, scal, out):
    from contextlib import ExitStack
    ctx = ExitStack()
    stage_pool = ctx.enter_context(tc.tile_pool(name="stage", bufs=2))
    wsx_pool = ctx.enter_context(tc.tile_pool(name="wsx", bufs=3))
    w_pool = ctx.enter_context(tc.tile_pool(name="wts", bufs=8))
    psum = ctx.enter_context(tc.tile_pool(name="psum", bufs=8, space="PSUM"))
    ract = ctx.enter_context(tc.tile_pool(name="ract", bufs=5))
    sact = ctx.enter_context(tc.tile_pool(name="sact", bufs=5))
    dtmp = ctx.enter_context(tc.tile_pool(name="dtmp", bufs=5))
    otp = ctx.enter_context(tc.tile_pool(name="ot", bufs=4))
    wtp = ctx.enter_context(tc.tile_pool(name="wt", bufs=1))
    tkp = ctx.enter_context(tc.tile_pool(name="tk", bufs=2))

    # --- small consts: diag taps + packed scalars first in the DMA queue
    dg = wtp.tile([128, WIDTH + 1, 128], FP16, tag="dg")
    nc.sync.dma_start(out=dg, in_=diagw)
    sc = wtp.tile([128, NSCAL], FP32, tag="sc")
    nc.sync.dma_start(out=sc, in_=scal)
    wt_t = {(l, d): sc[:, (l * 2 + d) * 5:(l * 2 + d) * 5 + 5]
            for l in range(L) for d in range(2)}
    bt_t = {(l, d, i): sc[:, 20 + ((l * 2 + d) * 2 + i) * NT:
                          20 + ((l * 2 + d) * 2 + i) * NT + NT]
            for l in range(L) for d in range(2) for i in range(NH)}

    # --- layer-0 staging (shared by both directions; pads baked in),
    # split per-kt, ordered so the PE-diag kts land first
    stage_l0 = stage_pool.tile([128, KT, BL, PSEQ], FP16, tag="stage")
    xs_v = xstage.rearrange("(kt p) bl s -> p kt bl s", p=128)
    for kt in range(KT):
        nc.sync.dma_start(out=stage_l0[:, kt], in_=xs_v[:, kt])

    def ws_tree_kt(stage_t, ws_t, wt, offs, kt):
        """DVE window sum for one kt: TS (4x mode) + TT-add (2x mode) tree."""
        src = lambda k: stage_t[:, kt, :, offs + k:offs + k + S]
        acc = ws_t[:, kt]
        nc.vector.tensor_scalar(acc, src(0), wt[:, 0:1], None, op0=OP.mult)
        for k in range(1, WIDTH + 1):
            tk = tkp.tile([128, BL, S], FP16, tag="tk")
            nc.vector.tensor_scalar(tk, src(k), wt[:, k:k + 1], None,
                                    op0=OP.mult)
            nc.vector.tensor_add(acc, acc, tk)

    def do_ws(stage_t, ws_t, l, d):
        wt = wt_t[(l, d)]
        offs = 0 if d == 0 else WIDTH
        for kt in range(KT):
            ws_tree_kt(stage_t, ws_t, wt, offs, kt)

    def do_ws_f0_pe(stage_t, ws_t):
        """Prologue fw window sum: kt < PE_WS_KT on PE as 5 accumulating
        diagonal matmuls per (kt, bl) unit (ldweights amortized across the
        8 in-flight PSUM banks), ACT copying PSUM -> fp16; remaining kts on
        the DVE tree."""
        units = [(kt, bl) for kt in range(PE_WS_KT) for bl in range(BL)]
        for g in range(0, len(units), 8):
            grp = units[g:g + 8]
            pss = [psum.tile([128, S], FP32, tag="ps", name=f"wsps{g}_{i}")
                   for i in range(len(grp))]
            for k in range(WIDTH + 1):
                nc.tensor.ldweights(dg[:, k])
                for ps, (kt, bl) in zip(pss, grp):
                    nc.tensor.matmul(ps, dg[:, k],
                                     stage_t[:, kt, bl, k:k + S],
                                     start=(k == 0), stop=(k == WIDTH))
            for ps, (kt, bl) in zip(pss, grp):
                nc.scalar.copy(ws_t[:, kt, bl], ps)
        wt = wt_t[(0, 0)]
        for kt in range(PE_WS_KT, KT):
            ws_tree_kt(stage_t, ws_t, wt, 0, kt)

    def pad_stage(stage_t):
        """DMA the layer-1 pad vectors into the staging tile edges."""
        for kt in range(KT):
            fp_ap = fwpad1[bass.ts(kt, 128), :]
            bp_ap = bwpad1[bass.ts(kt, 128), :]
            nc.sync.dma_start(
                out=stage_t[:, kt, :, 0:WIDTH],
                in_=bass.AP(tensor=fp_ap.tensor, offset=fp_ap.offset,
                            ap=[fp_ap.ap[0], [0, BL], fp_ap.ap[1]]))
            nc.sync.dma_start(
                out=stage_t[:, kt, :, WIDTH + S:PSEQ],
                in_=bass.AP(tensor=bp_ap.tensor, offset=bp_ap.offset,
                            ap=[bp_ap.ap[0], [0, BL], bp_ap.ap[1]]))

    def mm_block(ps_list, W, x_in):
        """k-outer / t-inner accumulation with explicit ldweights: each
        stationary [128,128] is loaded once and streamed over all 4 token
        blocks (183 ns/matmul on HW vs 276 self-loading)."""
        for k in range(KT):
            nc.tensor.ldweights(W[:, k])
            for t in range(BL):
                nc.tensor.matmul(ps_list[t], W[:, k], x_in[:, k, t],
                                 start=(k == 0), stop=(k == KT - 1))

    def do_highway(l, d, i, x_in, x0=None, stage_out=None, res_stage=None,
                   bg=None):
        """One highway sublayer. i==0 writes x0; i==1 writes either the
        layer-1 staging body (l==0) or ot tiles + residual (l==1), and DMAs
        the direction's output. Final combine adds run on Pool.

        bg: optional list of thunks; bg[jj] is emitted after pair jj's
        combines so background DVE work (the next window sum) interleaves
        with the combines instead of blocking them in the in-order queue."""
        bt = bt_t[(l, d, i)]
        pfx = f"{l}{d}{i}"
        for jj in range(KT):  # paired feature tiles: nl=jj, gate=jj+8
            Wnl = w_pool.tile([128, KT, 128], FP16, tag="W")
            nc.sync.dma_start(out=Wnl, in_=wts[d, l, i, jj])
            Wsg = w_pool.tile([128, KT, 128], FP16, tag="W")
            nc.sync.dma_start(out=Wsg, in_=wts[d, l, i, jj + KT])
            ps_nl = [psum.tile([128, S], FP32, tag="ps", name=f"psnl{pfx}_{jj}_{t}")
                     for t in range(BL)]
            mm_block(ps_nl, Wnl, x_in)
            ps_sg = [psum.tile([128, S], FP32, tag="ps", name=f"pssg{pfx}_{jj}_{t}")
                     for t in range(BL)]
            mm_block(ps_sg, Wsg, x_in)
            for t in range(BL):
                r = ract.tile([128, S], FP16, tag="r")
                nc.scalar.activation(r, ps_nl[t], AF.Relu, bias=bt[:, jj:jj + 1])
                sg = sact.tile([128, S], FP16, tag="sg")
                nc.scalar.activation(sg, ps_sg[t], AF.Sigmoid,
                                     bias=bt[:, jj + KT:jj + KT + 1])
                dd = dtmp.tile([128, S], FP16, tag="dd")
                nc.vector.tensor_sub(dd, x_in[:, jj, t], r)   # x - relu
                nc.vector.tensor_mul(dd, sg, dd)              # g*(x - relu)
                # out = g*x + (1-g)*relu = g*(x-relu) + relu
                if x0 is not None:
                    nc.gpsimd.tensor_add(x0[:, jj, t], dd, r)
                elif stage_out is not None:
                    body = stage_out[:, jj, t, WIDTH:WIDTH + S]
                    nc.gpsimd.tensor_add(body, dd, r)
                    nc.sync.dma_start(out=out[l, d, bass.ts(jj, 128), t, :],
                                      in_=body)
                else:
                    # final sublayers: adds stay on DVE (it is idle here and
                    # ~3x faster per op, shrinking the pipeline tail)
                    res = res_stage[:, jj, t, WIDTH:WIDTH + S]
                    nc.vector.tensor_add(dd, dd, r)
                    ot = otp.tile([128, S], FP16, tag="ot")
                    nc.vector.tensor_add(ot, dd, res)         # + layer input
                    nc.sync.dma_start(out=out[l, d, bass.ts(jj, 128), t, :],
                                      in_=ot)
            if bg is not None and jj < len(bg):
                bg[jj]()

    # ---------------- layer 0 ----------------
    ws_f0 = wsx_pool.tile([128, KT, BL, S], FP16, tag="wsx")
    do_ws_f0_pe(stage_l0, ws_f0)
    ws_b0 = wsx_pool.tile([128, KT, BL, S], FP16, tag="wsx")

    x0_f = wsx_pool.tile([128, KT, BL, S], FP16, tag="wsx")
    wtb0 = wt_t[(0, 1)]
    do_highway(0, 0, 0, ws_f0, x0=x0_f,
               bg=[(lambda kt=kt: ws_tree_kt(stage_l0, ws_b0, wtb0, WIDTH, kt))
                   for kt in range(KT)])
    x0_b = wsx_pool.tile([128, KT, BL, S], FP16, tag="wsx")
    do_highway(0, 1, 0, ws_b0, x0=x0_b)

    stage_f = stage_pool.tile([128, KT, BL, PSEQ], FP16, tag="stage")
    pad_stage(stage_f)
    do_highway(0, 0, 1, x0_f, stage_out=stage_f)

    # layer-1 fw window sum: emitted here so it runs on DVE under h1_b's
    # matmuls (stage_f's body is complete by then)
    ws_f1 = wsx_pool.tile([128, KT, BL, S], FP16, tag="wsx")
    do_ws(stage_f, ws_f1, 1, 0)

    stage_b = stage_pool.tile([128, KT, BL, PSEQ], FP16, tag="stage")
    pad_stage(stage_b)
    do_highway(0, 1, 1, x0_b, stage_out=stage_b)

    ws_b1 = wsx_pool.tile([128, KT, BL, S], FP16, tag="wsx")
    do_ws(stage_b, ws_b1, 1, 1)

    # ---------------- layer 1 ----------------
    x0_f1 = wsx_pool.tile([128, KT, BL, S], FP16, tag="wsx")
    do_highway(1, 0, 0, ws_f1, x0=x0_f1)
    x0_b1 = wsx_pool.tile([128, KT, BL, S], FP16, tag="wsx")
    do_highway(1, 1, 0, ws_b1, x0=x0_b1)

    do_highway(1, 0, 1, x0_f1, res_stage=stage_f)
    do_highway(1, 1, 1, x0_b1, res_stage=stage_b)

    ctx.close()


def prepare_in_maps(inputs, fw_pad, bw_pad, fw_w, bw_w,
                    fw_hw_W, fw_hw_b, bw_hw_W, bw_hw_b):
    """Shard + lay out host-side. Returns list of 8 per-core input dicts."""
    fw_pad = np.asarray(fw_pad, dtype=np.float32)
    bw_pad = np.asarray(bw_pad, dtype=np.float32)

    # weights: [L, NH, H, 2H] -> [L, NH, NT, 128p, KT, 128c] fp16 per dir
    def tile_w(Wd):
        Wd = np.asarray(Wd, dtype=np.float32).reshape(L, NH, KT, 128, NT, 128)
        return np.ascontiguousarray(
            Wd.transpose(0, 1, 4, 3, 2, 5)).astype(np.float16)

    wts = np.stack([tile_w(fw_hw_W), tile_w(bw_hw_W)], axis=0)

    # packed scalars blob [128, NSCAL]: taps then biases (see build_nc)
    scal = np.zeros((128, NSCAL), dtype=np.float32)
    w_by_dir = (np.asarray(fw_w, np.float32), np.asarray(bw_w, np.float32))
    b_by_dir = (np.asarray(fw_hw_b, np.float32), np.asarray(bw_hw_b, np.float32))
    for l in range(L):
        for d in range(2):
            base = (l * 2 + d) * 5
            scal[:, base:base + 5] = w_by_dir[d][l][None, :]
            for i in range(NH):
                bb = 20 + ((l * 2 + d) * 2 + i) * NT
                scal[:, bb:bb + NT] = b_by_dir[d][l, i].reshape(NT, 128).T

    # diag(w_k) [128, 5, 128] fp16 for the layer-0 fw taps (PE window sum)
    diagw = np.zeros((128, WIDTH + 1, 128), dtype=np.float16)
    for k in range(WIDTH + 1):
        np.fill_diagonal(diagw[:, k, :], np.float16(w_by_dir[0][0, k]))

    common = {
        "wts": wts,
        "fwpad1": np.ascontiguousarray(fw_pad[1].T).astype(np.float16),
        "bwpad1": np.ascontiguousarray(bw_pad[1].T).astype(np.float16),
        "diagw": diagw,
        "scal": scal,
    }

    # layer-0 padded staging [H, BL, PSEQ] fp16: [fw_pad[0]; x; bw_pad[0]]
    x = np.asarray(inputs, dtype=np.float32)
    in_maps = []
    for c in range(CORES):
        shard = x[c * BL:(c + 1) * BL]                       # [BL, S, H]
        x_feat = np.transpose(shard, (2, 0, 1))              # [H, BL, S]
        stg = np.empty((H, BL, PSEQ), dtype=np.float16)
        stg[:, :, WIDTH:WIDTH + S] = x_feat.astype(np.float16)
        stg[:, :, 0:WIDTH] = fw_pad[0].T.astype(np.float16)[:, None, :]
        stg[:, :, WIDTH + S:] = bw_pad[0].T.astype(np.float16)[:, None, :]
        in_maps.append({"xstage": stg, **common})
    return in_maps


def assemble_output(results):
    """Per-core out [L, 2, H, BL, S] fp16 -> full [L, B, S, 2H] fp32."""
    full = np.empty((L, B, S, 2 * H), dtype=np.float32)
    for c, r in enumerate(results):
        o = np.asarray(r["out"], dtype=np.float32)  # [L, 2, H, BL, S]
        full[:, c * BL:(c + 1) * BL] = np.transpose(
            o, (0, 3, 4, 1, 2)).reshape(L, BL, S, 2 * H)
    return full


_NC_CACHE = None


def kernel(inputs, masks, fw_pad, bw_pad, fw_w, bw_w,
           fw_hw_W, fw_hw_b, bw_hw_W, bw_hw_b):
    del masks  # all-ones; unused by the reference computation
    global _NC_CACHE
    if _NC_CACHE is None:
        _NC_CACHE = build_nc()
    in_maps = prepare_in_maps(inputs, fw_pad, bw_pad, fw_w, bw_w,
                              fw_hw_W, fw_hw_b, bw_hw_W, bw_hw_b)
    res = bass_utils.run_bass_kernel_spmd(_NC_CACHE, in_maps,
                                          core_ids=list(range(CORES)))
    return assemble_output(res.results)


if __name__ == "__main__":
    nc = build_nc()
    print("built ok")
